# revision 3
# baseline (speedup 1.0000x reference)
"""Masked attention (B=4, M=N=4096, D=64) on 8 Trainium2 NeuronCores.

Sharding: batch (4) x m-halves (2) -> 8 cores, no cross-core communication.
Each core computes out[m, :] = softmax(mask(q@k^T)/sqrt(d)) @ v for its
2048 q rows against the full 4096 k/v rows of its batch.

Device algorithm (per core), designed around engine rooflines:
  - Scores are computed TRANSPOSED: S^T[n, m] = (kT chunk).T @ qT, so the
    attention-weight matrix is produced with n (the PV contraction dim) on
    partitions -- no transpose of the big attention matrix is ever needed.
    q and k are shipped pre-transposed ([d, m] / [d, n]) from the host.
  - ScalarE computes e = exp(S^T * 1/sqrt(d) - 3) straight out of PSUM
    (the -3 shift cancels in softmax normalization; it keeps e < 255).
  - VectorE applies the mask: p = e * notmaskT (bf16 tensor_tensor, 2x mode).
  - PV: out^T[j, m] += v_aug_chunk.T @ p, where v_aug = [v | ones]; the ones
    column makes row 64 of out^T the softmax denominator l[m] for free.
  - Finalize: PE-transpose out^T in 128-col blocks, reciprocal of l,
    per-partition scale, DMA out.
  - QK matmuls have K=64 (= d), so consecutive n-chunks are packed into
    row-halves of the PE array (tile_position via base_partition 0/64) and
    overlap on hardware.
"""

import numpy as np
import ml_dtypes
from contextlib import ExitStack

import concourse.bacc as bacc
import concourse.mybir as mybir
import concourse.tile as tile
from concourse.bass_utils import run_bass_kernel_spmd
from concourse.masks import make_identity

B, M, N, D = 4, 4096, 4096, 64
NCORES = 8
M_LOC = M // 2        # q rows per core
MH = 1024             # m sub-block held in one PSUM accumulation
NCH = N // 128        # 32 n-chunks of 128
SCALE = 1.0 / 8.0     # 1/sqrt(64)
EBIAS = -3.0
BF16 = mybir.dt.bfloat16
F32 = mybir.dt.float32
BF = ml_dtypes.bfloat16

_NC = None
LAST_RESULTS = None   # BassKernelResults of the most recent run (for profiling)
TRACE = False
TRACE_KW = {}


def _build_nc():
    nc = bacc.Bacc("TRN2", target_bir_lowering=False, debug=False,
                   num_devices=NCORES)
    qT = nc.dram_tensor("qT", [128, M_LOC], BF16, kind="ExternalInput").ap()
    kT = nc.dram_tensor("kT", [128, (NCH // 2) * 128], BF16,
                        kind="ExternalInput").ap()
    vA = nc.dram_tensor("vA", [128, NCH * (D + 1)], BF16,
                        kind="ExternalInput").ap()
    nmT = nc.dram_tensor("nmT", [N, M_LOC], BF16, kind="ExternalInput").ap()
    o = nc.dram_tensor("o", [M_LOC, D], F32, kind="ExternalOutput").ap()

    with tile.TileContext(nc) as tc, ExitStack() as ctx:
        const = ctx.enter_context(tc.tile_pool(name="const", bufs=1))
        mpool = ctx.enter_context(tc.tile_pool(name="mask", bufs=6))
        epool = ctx.enter_context(tc.tile_pool(name="e", bufs=4))
        ppool = ctx.enter_context(tc.tile_pool(name="p", bufs=4))
        fpool = ctx.enter_context(tc.tile_pool(name="fin", bufs=2))
        small = ctx.enter_context(tc.tile_pool(name="small", bufs=3))
        spool = ctx.enter_context(tc.tile_pool(name="spsum", bufs=2, space="PSUM"))
        opool = ctx.enter_context(tc.tile_pool(name="opsum", bufs=1, space="PSUM"))
        tpool = ctx.enter_context(tc.tile_pool(name="tpsum", bufs=2, space="PSUM"))

        qT_s = const.tile([128, M_LOC], BF16)
        nc.sync.dma_start(qT_s[:], qT)
        kT_s = const.tile([128, (NCH // 2) * 128], BF16)
        nc.sync.dma_start(kT_s[:], kT)
        vA_s = const.tile([128, NCH * (D + 1)], BF16)
        nc.sync.dma_start(vA_s[:], vA)
        ident = const.tile([128, 128], F32)
        make_identity(nc, ident[:])
        ebias = const.tile([128, 1], F32)
        nc.vector.memset(ebias[:], EBIAS)

        # ~5us of dense back-to-back matmuls to flip the PE HAM clock-gate
        # to 8/8 before the real work starts (results discarded).
        wu = tpool.tile([128, 512], F32, tag="t")
        for _ in range(14):
            nc.tensor.matmul(wu[:], kT_s[0:64, 0:128], kT_s[0:64, 0:512],
                             start=True, stop=True)

        for h in range(2):
            o_ps = opool.tile([D + 1, MH], F32)
            for pc in range(NCH // 2):
                ni_e, ni_o = 2 * pc, 2 * pc + 1
                lhs_e = kT_s[0:64, pc * 128:(pc + 1) * 128]
                lhs_o = kT_s[64:128, pc * 128:(pc + 1) * 128]
                rhs_e = qT_s[0:64, h * MH:(h + 1) * MH]
                rhs_o = qT_s[64:128, h * MH:(h + 1) * MH]
                S_e = spool.tile([128, MH], F32, tag="s")
                S_o = spool.tile([128, MH], F32, tag="s")
                # interleave row-halves so consecutive MMs overlap on the PE
                nc.tensor.matmul(S_e[:, 0:512], lhs_e, rhs_e[:, 0:512],
                                 start=True, stop=True)
                nc.tensor.matmul(S_o[:, 0:512], lhs_o, rhs_o[:, 0:512],
                                 start=True, stop=True)
                nc.tensor.matmul(S_e[:, 512:1024], lhs_e, rhs_e[:, 512:1024],
                                 start=True, stop=True)
                nc.tensor.matmul(S_o[:, 512:1024], lhs_o, rhs_o[:, 512:1024],
                                 start=True, stop=True)
                for ni, S in ((ni_e, S_e), (ni_o, S_o)):
                    e = epool.tile([128, MH], BF16)
                    nc.scalar.activation(e[:], S[:],
                                         mybir.ActivationFunctionType.Exp,
                                         bias=ebias[:], scale=SCALE)
                    nm = mpool.tile([128, MH], BF16)
                    nc.sync.dma_start(nm[:], nmT[ni * 128:(ni + 1) * 128,
                                                 h * MH:(h + 1) * MH])
                    p = ppool.tile([128, MH], BF16)
                    nc.vector.tensor_mul(p[:], e[:], nm[:])
                    vch = vA_s[:, ni * (D + 1):(ni + 1) * (D + 1)]
                    nc.tensor.matmul(o_ps[:, 0:512], vch, p[:, 0:512],
                                     start=(ni == 0), stop=(ni == NCH - 1))
                    nc.tensor.matmul(o_ps[:, 512:1024], vch, p[:, 512:1024],
                                     start=(ni == 0), stop=(ni == NCH - 1))
            oT = fpool.tile([D + 1, MH], F32)
            nc.vector.tensor_copy(oT[:], o_ps[:])
            for mc in range(MH // 128):
                t_ps = tpool.tile([128, D + 1], F32, tag="t")
                nc.tensor.transpose(t_ps[:], oT[:, mc * 128:(mc + 1) * 128],
                                    ident[0:D + 1, 0:D + 1])
                rl = small.tile([128, 1], F32)
                nc.vector.reciprocal(rl[:], t_ps[:, D:D + 1])
                og = small.tile([128, D], F32)
                nc.vector.tensor_scalar_mul(og[:], t_ps[:, 0:D], rl[:])
                nc.sync.dma_start(o[h * MH + mc * 128: h * MH + (mc + 1) * 128, :],
                                  og[:])
    nc.compile()
    return nc


def _get_nc():
    global _NC
    if _NC is None:
        _NC = _build_nc()
    return _NC


def _prep_core(q, k, v, mask, b, j):
    qs = q[b, j * M_LOC:(j + 1) * M_LOC, :]
    qT = np.ascontiguousarray(qs.T).astype(BF)            # [64, 2048]
    qTp = np.concatenate([qT, qT], axis=0)                # [128, 2048]
    kTf = np.ascontiguousarray(k[b].T).astype(BF)         # [64, 4096]
    kTp = np.empty((128, (NCH // 2) * 128), BF)
    kTr = kTf.reshape(64, NCH, 128)
    kTp[0:64] = kTr[:, 0::2, :].reshape(64, -1)
    kTp[64:128] = kTr[:, 1::2, :].reshape(64, -1)
    vb = v[b]                                             # [4096, 64]
    vA = np.empty((128, NCH * (D + 1)), BF)
    vAr = vA.reshape(128, NCH, D + 1)
    vAr[:, :, :D] = vb.reshape(NCH, 128, D).transpose(1, 0, 2).astype(BF)
    vAr[:, :, D] = np.asarray(1.0, BF)
    nmT = np.ascontiguousarray(
        (~mask[b, j * M_LOC:(j + 1) * M_LOC, :]).T).astype(BF)  # [4096, 2048]
    return {"qT": qTp, "kT": kTp, "vA": vA, "nmT": nmT}


def kernel(q, k, v, mask):
    global LAST_RESULTS
    q = np.asarray(q, dtype=np.float32)
    k = np.asarray(k, dtype=np.float32)
    v = np.asarray(v, dtype=np.float32)
    mask = np.asarray(mask)
    nc = _get_nc()
    in_maps = [_prep_core(q, k, v, mask, c // 2, c % 2) for c in range(NCORES)]
    res = run_bass_kernel_spmd(nc, in_maps, core_ids=list(range(NCORES)),
                               trace=TRACE, **TRACE_KW)
    LAST_RESULTS = res
    out = np.empty((B, M, D), np.float32)
    for c in range(NCORES):
        b, j = divmod(c, 2)
        out[b, j * M_LOC:(j + 1) * M_LOC, :] = res.results[c]["o"]
    return out


# revision 8
# speedup vs baseline: 1.1828x; 1.1828x over previous
"""Masked attention (B=4, M=N=4096, D=64) on 8 Trainium2 NeuronCores.

Sharding: batch (4) x m-halves (2) -> 8 cores, no cross-core communication.
Each core computes out[m, :] = softmax(mask(q@k^T)/sqrt(d)) @ v for its
2048 q rows against the full 4096 k/v rows of its batch.

Device algorithm (per core), designed around engine rooflines:
  - Scores are computed TRANSPOSED: S^T[n, m] = (kT chunk).T @ qT, so the
    attention-weight matrix is produced with n (the PV contraction dim) on
    partitions -- no transpose of the big attention matrix is ever needed.
    q and k are shipped pre-transposed ([d, m] / [d, n]) from the host.
  - ScalarE computes e = exp(S^T * 1/sqrt(d) - 3) straight out of PSUM
    (the -3 shift cancels in softmax normalization; it keeps e < 255).
  - VectorE applies the mask: p = e * notmaskT (bf16 tensor_tensor, 2x mode).
  - PV: out^T[j, m] += v_aug_chunk.T @ p, where v_aug = [v | ones]; the ones
    column makes row 64 of out^T the softmax denominator l[m] for free.
  - Finalize: PE-transpose out^T in 128-col blocks, reciprocal of l,
    per-partition scale, DMA out.
  - QK matmuls have K=64 (= d), so consecutive n-chunks are packed into
    row-halves of the PE array (tile_position via base_partition 0/64) and
    overlap on hardware.
"""

import numpy as np
import ml_dtypes
from contextlib import ExitStack

import concourse.bacc as bacc
import concourse.mybir as mybir
import concourse.tile as tile
from concourse.bass_utils import run_bass_kernel_spmd
from concourse.masks import make_identity

B, M, N, D = 4, 4096, 4096, 64
NCORES = 8
M_LOC = M // 2        # q rows per core
MH = 1024             # m sub-block held in one PSUM accumulation
NCH = N // 128        # 32 n-chunks of 128
SCALE = 1.0 / 8.0     # 1/sqrt(64)
EBIAS = -3.0
BF16 = mybir.dt.bfloat16
F32 = mybir.dt.float32
BF = ml_dtypes.bfloat16

_NC = None
LAST_RESULTS = None   # BassKernelResults of the most recent run (for profiling)
TRACE = False
TRACE_KW = {}


def _build_nc():
    nc = bacc.Bacc("TRN2", target_bir_lowering=False, debug=False,
                   num_devices=NCORES)
    qT = nc.dram_tensor("qT", [128, M_LOC], BF16, kind="ExternalInput").ap()
    kT = nc.dram_tensor("kT", [128, (NCH // 2) * 128], BF16,
                        kind="ExternalInput").ap()
    vA = nc.dram_tensor("vA", [128, NCH * (D + 1)], BF16,
                        kind="ExternalInput").ap()
    nmT = nc.dram_tensor("nmT", [N, M_LOC], BF16, kind="ExternalInput").ap()
    o = nc.dram_tensor("o", [M_LOC, D], F32, kind="ExternalOutput").ap()

    with tile.TileContext(nc) as tc, ExitStack() as ctx:
        const = ctx.enter_context(tc.tile_pool(name="const", bufs=1))
        mpool = ctx.enter_context(tc.tile_pool(name="mask", bufs=4))
        epool = ctx.enter_context(tc.tile_pool(name="e", bufs=4))
        ppool = ctx.enter_context(tc.tile_pool(name="p", bufs=4))
        fpool = ctx.enter_context(tc.tile_pool(name="fin", bufs=2))
        small = ctx.enter_context(tc.tile_pool(name="small", bufs=3))
        spool = ctx.enter_context(tc.tile_pool(name="spsum", bufs=2, space="PSUM"))
        opool = ctx.enter_context(tc.tile_pool(name="opsum", bufs=1, space="PSUM"))
        tpool = ctx.enter_context(tc.tile_pool(name="tpsum", bufs=2, space="PSUM"))

        qT_s = const.tile([128, M_LOC], BF16)
        nc.sync.dma_start(qT_s[:], qT)
        kT_s = const.tile([128, (NCH // 2) * 128], BF16)
        nc.sync.dma_start(kT_s[:], kT)
        vA_s = const.tile([128, NCH * (D + 1)], BF16)
        nc.sync.dma_start(vA_s[:], vA)
        ident = const.tile([128, 128], F32)
        make_identity(nc, ident[:])
        ebias = const.tile([128, 1], F32)
        nc.vector.memset(ebias[:], EBIAS)

        # ~5us of dense back-to-back full-array (K=128) matmuls to flip the
        # PE HAM clock-gate to 8/8 before the real work starts (results
        # discarded). K=64 matmuls do NOT trip the activity monitor.
        wu = tpool.tile([128, 512], F32, tag="t")
        for _ in range(16):
            nc.tensor.matmul(wu[:], kT_s[:, 0:128], kT_s[:, 512:1024],
                             start=True, stop=True)

        for h in range(2):
            o_ps = opool.tile([D + 1, MH], F32)
            for pc in range(NCH // 2):
                ni_e, ni_o = 2 * pc, 2 * pc + 1
                lhs_e = kT_s[0:64, pc * 128:(pc + 1) * 128]
                lhs_o = kT_s[64:128, pc * 128:(pc + 1) * 128]
                rhs_e = qT_s[0:64, h * MH:(h + 1) * MH]
                rhs_o = qT_s[64:128, h * MH:(h + 1) * MH]
                S_e = spool.tile([128, MH], F32, tag="s")
                S_o = spool.tile([128, MH], F32, tag="s")
                # interleave row-halves so consecutive MMs overlap on the PE
                nc.tensor.matmul(S_e[:, 0:512], lhs_e, rhs_e[:, 0:512],
                                 start=True, stop=True)
                nc.tensor.matmul(S_o[:, 0:512], lhs_o, rhs_o[:, 0:512],
                                 start=True, stop=True)
                nc.tensor.matmul(S_e[:, 512:1024], lhs_e, rhs_e[:, 512:1024],
                                 start=True, stop=True)
                nc.tensor.matmul(S_o[:, 512:1024], lhs_o, rhs_o[:, 512:1024],
                                 start=True, stop=True)
                # one DMA for the pair's mask rows: [256, MH] -> [128, 2*MH]
                nm = mpool.tile([128, 2 * MH], BF16)
                nm_src = nmT[ni_e * 128:(ni_e + 2) * 128,
                             h * MH:(h + 1) * MH].rearrange(
                                 "(t p) m -> p t m", t=2)
                nc.sync.dma_start(nm[:].rearrange("p (t m) -> p t m", t=2),
                                  nm_src)
                for half, (ni, S) in enumerate(((ni_e, S_e), (ni_o, S_o))):
                    e = epool.tile([128, MH], BF16)
                    nc.scalar.activation(e[:], S[:],
                                         mybir.ActivationFunctionType.Exp,
                                         bias=ebias[:], scale=SCALE)
                    p = ppool.tile([128, MH], BF16)
                    nc.vector.tensor_mul(p[:], e[:],
                                         nm[:, half * MH:(half + 1) * MH])
                    vch = vA_s[:, ni * (D + 1):(ni + 1) * (D + 1)]
                    nc.tensor.matmul(o_ps[:, 0:512], vch, p[:, 0:512],
                                     start=(ni == 0), stop=(ni == NCH - 1))
                    nc.tensor.matmul(o_ps[:, 512:1024], vch, p[:, 512:1024],
                                     start=(ni == 0), stop=(ni == NCH - 1))
            oT = fpool.tile([D + 1, MH], F32)
            nc.vector.tensor_copy(oT[:], o_ps[:])
            for mc in range(MH // 128):
                t_ps = tpool.tile([128, D + 1], F32, tag="t")
                nc.tensor.transpose(t_ps[:], oT[:, mc * 128:(mc + 1) * 128],
                                    ident[0:D + 1, 0:D + 1])
                rl = small.tile([128, 1], F32)
                nc.vector.reciprocal(rl[:], t_ps[:, D:D + 1])
                og = small.tile([128, D], F32)
                nc.vector.tensor_scalar_mul(og[:], t_ps[:, 0:D], rl[:])
                nc.gpsimd.dma_start(o[h * MH + mc * 128: h * MH + (mc + 1) * 128, :],
                                    og[:])
    nc.compile()
    return nc


def _get_nc():
    global _NC
    if _NC is None:
        _NC = _build_nc()
    return _NC


def _prep_core(q, k, v, mask, b, j):
    qs = q[b, j * M_LOC:(j + 1) * M_LOC, :]
    qT = np.ascontiguousarray(qs.T).astype(BF)            # [64, 2048]
    qTp = np.concatenate([qT, qT], axis=0)                # [128, 2048]
    kTf = np.ascontiguousarray(k[b].T).astype(BF)         # [64, 4096]
    kTp = np.empty((128, (NCH // 2) * 128), BF)
    kTr = kTf.reshape(64, NCH, 128)
    kTp[0:64] = kTr[:, 0::2, :].reshape(64, -1)
    kTp[64:128] = kTr[:, 1::2, :].reshape(64, -1)
    vb = v[b]                                             # [4096, 64]
    vA = np.empty((128, NCH * (D + 1)), BF)
    vAr = vA.reshape(128, NCH, D + 1)
    vAr[:, :, :D] = vb.reshape(NCH, 128, D).transpose(1, 0, 2).astype(BF)
    vAr[:, :, D] = np.asarray(1.0, BF)
    nmT = np.ascontiguousarray(
        (~mask[b, j * M_LOC:(j + 1) * M_LOC, :]).T).astype(BF)  # [4096, 2048]
    return {"qT": qTp, "kT": kTp, "vA": vA, "nmT": nmT}


def kernel(q, k, v, mask):
    global LAST_RESULTS
    q = np.asarray(q, dtype=np.float32)
    k = np.asarray(k, dtype=np.float32)
    v = np.asarray(v, dtype=np.float32)
    mask = np.asarray(mask)
    nc = _get_nc()
    in_maps = [_prep_core(q, k, v, mask, c // 2, c % 2) for c in range(NCORES)]
    res = run_bass_kernel_spmd(nc, in_maps, core_ids=list(range(NCORES)),
                               trace=TRACE, **TRACE_KW)
    LAST_RESULTS = res
    out = np.empty((B, M, D), np.float32)
    for c in range(NCORES):
        b, j = divmod(c, 2)
        out[b, j * M_LOC:(j + 1) * M_LOC, :] = res.results[c]["o"]
    return out
